# revision 35
# baseline (speedup 1.0000x reference)
"""Trainium2 Bass kernel for nn_HadamardClassifier (self-contained).

Math: out = -scale * l2norm_rows(x) @ H + bias, with H the [2048, 14951]
top-left slice of the 16384x16384 Sylvester Hadamard matrix,
H[i, j] = (-1)^popcount(i & j).

Since row index i < 2048 uses only 11 bits, H[i, j] == H2048[i, j & 2047]:
the output is a periodic tiling of y = xn' @ H2048 (7.3x FLOP reduction).
Further, H2048 = H4 (x) H512 (Kronecker split at bit 9), so
    y[m, jH*512 + jl] = sum_iH H4[iH, jH] * z[m, iH, jl]
    z[m, iH, jl]      = sum_iL H512[iL, jl] * x'[m, iH*512 + iL]

Device dataflow (per core, 512 rows, 4 chunks of 128):
- x arrives PRE-TRANSPOSED and bf16-cast from the host (input marshalling),
  so stage 1 is a straight K=512 PE matmul against H512 panels -- no
  on-device transposes, no PSUM evac of transposes.
- Row norms come free via Parseval: ||z_row||^2 = 512 * ||x_row||^2, so an
  ACT square-accumulate over the PSUM z replaces separate x^2 passes.  The
  positive scale s = scale*sqrt(512)/||z|| is folded into the PSUM
  evacuation; the minus sign is folded into the fan-out by computing
  out = bias - y (tensor_tensor subtract) instead of y' + bias.
- Stage 2 is a 2-stage FWHT over iH on the vector engine in bf16.
- Fan-out: blocks 0-5 subtracted on DVE, block 6 + tail on GpSimd; outputs
  stream over both the SWDGE (gpsimd) and scalar-HWDGE DMA rings in bf16
  (the host upcasts to f32 during the gather -- halves HBM write traffic).
- PSUM z is double-buffered (8 banks) so the PE runs all 64 matmuls as one
  warm back-to-back burst (HAM clock-gate stays open).

Sharding: data-parallel over batch, 8 cores x 512 rows. No collectives.
"""

import numpy as np
import ml_dtypes

BATCH = 4096
IN_DIM = 2048
OUT_DIM = 14951
EPS = 1e-12
N_CORES = 8
M_PER_CORE = BATCH // N_CORES          # 512
N_CHUNKS = M_PER_CORE // 128           # 4 m-chunks of 128 rows
N_FULL_BLOCKS = OUT_DIM // IN_DIM      # 7
TAIL_COLS = OUT_DIM - N_FULL_BLOCKS * IN_DIM  # 615

NH = 4                                 # FWHT size (H4)
NL = IN_DIM // NH                      # 512 = matmul contraction dim
NP = NL // 128                         # 4 128-row panels per iH group

N_B_DVE = 6                            # blocks subtracted on DVE
DVE_COLS = N_B_DVE * IN_DIM            # cols [0, 12288)
GP_COLS = OUT_DIM - DVE_COLS           # cols [12288, 14951): block 6 + tail


def _hadamard(n):
    """Sylvester Hadamard matrix H[i,j] = (-1)^popcount(i&j), float32."""
    i = np.arange(n, dtype=np.uint32)[:, None]
    j = np.arange(n, dtype=np.uint32)[None, :]
    v = i & j
    pc = np.zeros_like(v)
    for b in range(int(n).bit_length()):
        pc += (v >> b) & 1
    return (1.0 - 2.0 * (pc & 1)).astype(np.float32)


def _patch_tile_drain():
    """This walrus build accepts only ONE sync-wait per instruction, but
    Tile's kernel-tail drain attaches the whole global clock to a single
    Drain ('Too many sync wait commands').  Split the waits onto a chain of
    single-wait sequencer nops instead."""
    import concourse.mybir as mybir
    import concourse.tile as tile
    from concourse.vector_clock import ScopedClock

    if getattr(tile.TileContext, "_drain_split_patched", False):
        return

    def _drain_and_barrier(self, tick_clock, wait_clock):
        nc = self.nc
        probe = nc.sync.nop()
        wait_clock.add_sem_waits(
            probe.ins, ScopedClock({None: tick_clock.global_clock})
        )
        si = probe.ins.sync_info
        waits = list(si.on_wait) if si is not None and si.on_wait else []
        if len(waits) > 1:
            si.on_wait = waits[:1]
            for w in waits[1:]:
                n = nc.sync.nop()
                n.ins.sync_info = mybir.SyncInfo(on_wait=[w], on_update=[])
        nc.sync.drain()
        nc.all_engine_barrier()
        assert self.sems is not None
        popped = nc._tile_sem_poison_stack.pop()
        assert popped is self._sem_poison
        nc.clear_and_free_semaphores(list(self.sems.allocated().values()))
        nc.all_engine_barrier()

    tile.TileContext._drain_and_barrier = _drain_and_barrier
    tile.TileContext._drain_split_patched = True


def _split_multiwait_instructions(nc):
    """This walrus build rejects instructions with more than one sync-wait.
    Hoist extra waits onto same-engine nop instructions inserted just before
    the offending instruction (engine queues execute in order, so waiting on
    the nops first is equivalent)."""
    import concourse.mybir as mybir

    n_split = 0
    for blk in nc.m.functions[0].blocks:
        new_list = []
        for inst in blk.instructions:
            si = inst.sync_info
            waits = list(si.on_wait) if si is not None and si.on_wait else []
            if len(waits) > 1:
                for k, w in enumerate(waits[:-1]):
                    nop = mybir.InstNoOp(
                        name=f"{inst.name}-wsplit{k}", ins=[], outs=[])
                    nop.engine = inst.engine
                    nop.sync_info = mybir.SyncInfo(on_wait=[w], on_update=[])
                    new_list.append(nop)
                    n_split += 1
                si.on_wait = waits[-1:]
            new_list.append(inst)
        blk.instructions = new_list
    return n_split


def _build_program():
    import concourse.bass as bass
    import concourse.mybir as mybir
    import concourse.tile as tile

    _patch_tile_drain()
    f32 = mybir.dt.float32
    bf16 = mybir.dt.bfloat16
    nc = bass.Bass()

    # x pre-transposed on the host: xt[kp, c, p, m] = x[c*128+m, p*128+kp]
    xt_d = nc.dram_tensor("xt", [128, N_CHUNKS, 16, 128], bf16,
                          kind="ExternalInput")
    hl_d = nc.dram_tensor("hl", [128, NP, NL], bf16, kind="ExternalInput")
    # bias pre-broadcast to 128 partitions on the host (stride-0 HBM
    # broadcast DMAs crawl at half rate)
    biasb_d = nc.dram_tensor("biasb", [128, OUT_DIM], bf16, kind="ExternalInput")
    # sqrt-input affine: t = sqrt(ss*sqc[0] + sqc[1]) = ||x||/scale, sigma = 1/t
    sqc_d = nc.dram_tensor("sqc", [128, 2], f32, kind="ExternalInput")
    # out in bf16: halves the dominant HBM write traffic; host upcasts to
    # f32 during the gather.
    out_d = nc.dram_tensor("out", [M_PER_CORE, OUT_DIM], bf16,
                           kind="ExternalOutput")

    from contextlib import ExitStack

    with tile.TileContext(nc) as tc, ExitStack() as ctx:
        singles = ctx.enter_context(tc.tile_pool(name="singles", bufs=1))
        scrpool = ctx.enter_context(tc.tile_pool(name="scr", bufs=2))
        fwpool = ctx.enter_context(tc.tile_pool(name="fw", bufs=4))
        opool = ctx.enter_context(tc.tile_pool(name="o", bufs=2))
        z_ps = ctx.enter_context(tc.tile_pool(name="z_ps", bufs=2, space="PSUM"))

        # --- all input loads up-front, issue-parallel across both HWDGE
        # rings, ordered by first use (per-ring DMA completions serialize
        # at ~2.3 us apiece, so each ring carries a short ordered list)
        xts = [None] * N_CHUNKS
        for c in range(N_CHUNKS):
            xts[c] = singles.tile([128, 16, 128], bf16, name=f"xt{c}")
            nc.sync.dma_start(out=xts[c], in_=xt_d[:, c, :, :])
        hl_s = singles.tile([128, NP, NL], bf16)
        nc.scalar.dma_start(out=hl_s, in_=hl_d[:, :, :])
        sqc_s = singles.tile([128, 2], f32)
        nc.scalar.dma_start(out=sqc_s, in_=sqc_d[:, :])
        # bias in 4 separate tiles so consumers only wait for their piece;
        # issues split across the sync and scalar rings
        bias_t = []
        bias_bounds = [0, 4096, 8192, 12288, OUT_DIM]
        for k in range(4):
            lo, hi = bias_bounds[k], bias_bounds[k + 1]
            t = singles.tile([128, hi - lo], bf16, name=f"bias{k}")
            eng = nc.sync if k % 2 == 0 else nc.scalar
            eng.dma_start(out=t, in_=biasb_d[:, lo:hi])
            bias_t.append(t)

        def bias_ap(lo, hi):
            """AP into the bias tiles for global column range [lo, hi)."""
            k = lo // 4096
            base = bias_bounds[k]
            assert hi <= bias_bounds[k + 1]
            return bias_t[k][:, lo - base:hi - base]

        # --- PE prewarm: the engines release from the runtime preamble ~7 us
        # before the first x chunk lands; a few dummy matmuls in that dead
        # window open the HAM clock gate so chunk 0 runs at 2.4 GHz.  They
        # write into the first PSUM buffer, which the first real iH group
        # overwrites (start=True).
        warm_sb = singles.tile([128, 512], bf16)
        nc.vector.memset(warm_sb, 0.0)
        # z in two half-tiles per chunk (iH groups 0-1 / 2-3) so the norm
        # square and the evacuation of the low half overlap the high half's
        # matmuls (Tile dependencies are tile-granular)
        zps = [(z_ps.tile([128, 2 * NL], f32, name="zpl", tag="zpl"),
                z_ps.tile([128, 2 * NL], f32, name="zph", tag="zph"))
               for _ in range(N_CHUNKS)]
        # dummies target chunk 1's tile (WAW never stalls chunk 0)
        for w in range(8):
            nc.tensor.matmul(
                zps[1][0][:, :NL], lhsT=warm_sb[:, :128], rhs=warm_sb,
                start=True, stop=(w == 7))

        for c in range(N_CHUNKS):
            rows = slice(c * 128, (c + 1) * 128)
            zpl, zph = zps[c]

            # --- stage 1: z[m, iH*512+jl] = sum_iL x'[m, iH*512+iL] H512[iL, jl]
            for iH in range(NH):
                zp_half = zpl if iH < 2 else zph
                for p in range(NP):
                    nc.tensor.matmul(
                        zp_half[:, (iH % 2) * NL:(iH % 2 + 1) * NL],
                        lhsT=xts[c][:, iH * NP + p, :], rhs=hl_s[:, p, :],
                        start=(p == 0), stop=(p == NP - 1))

            # --- row norms via Parseval: ss = sum(z^2) = 512*sum(x^2).
            # t = sqrt(ss*sqc0 + sqc1) = ||x||/scale (sqc0 = 1/(512*scale^2)),
            # sigma = 1/t = scale/||x||  (positive; sign folded into subtract).
            # Low-half square overlaps the high half's matmuls.
            sqz = scrpool.tile([128, NH * NL], bf16, tag="sqz")
            ssh = scrpool.tile([128, 2], f32, tag="ssh")
            nc.scalar.activation(
                out=sqz[:, :2 * NL], in_=zpl,
                func=mybir.ActivationFunctionType.Square,
                accum_out=ssh[:, 0:1])
            nc.scalar.activation(
                out=sqz[:, 2 * NL:], in_=zph,
                func=mybir.ActivationFunctionType.Square,
                accum_out=ssh[:, 1:2])
            ss = scrpool.tile([128, 1], f32, tag="ss")
            nc.scalar.add(ss, ssh[:, 0:1], ssh[:, 1:2])
            sig = scrpool.tile([128, 1], f32, tag="sig")
            nc.scalar.activation(
                out=sig, in_=ss, func=mybir.ActivationFunctionType.Sqrt,
                scale=sqc_s[:, 0:1], bias=sqc_s[:, 1:2])
            nc.vector.reciprocal(out=sig, in_=sig)

            # --- evacuate PSUM with the positive row scale, cast bf16.
            # Halves evac separately so FWHT stage 1's first butterfly
            # (which pairs iH 0 and 1 = the low half) starts after evac_lo.
            zw0 = fwpool.tile([128, NH, NL], bf16, tag="zw0")
            zw1 = fwpool.tile([128, NH, NL], bf16, tag="zw1")
            zw0f = zw0.rearrange("p a b -> p (a b)")
            nc.scalar.activation(
                out=zw0f[:, :2 * NL], in_=zpl,
                func=mybir.ActivationFunctionType.Copy, scale=sig)
            nc.scalar.activation(
                out=zw0f[:, 2 * NL:], in_=zph,
                func=mybir.ActivationFunctionType.Copy, scale=sig)

            # --- stage 2: FWHT over iH (dim 1), 2 butterfly stages on DVE ---
            cur, nxt = zw0, zw1
            for s in range(NH.bit_length() - 1):
                t = 1 << s
                cv = cur.rearrange("p (g two t) jl -> p g two t jl", two=2, t=t)
                nv = nxt.rearrange("p (g two t) jl -> p g two t jl", two=2, t=t)
                nc.vector.tensor_add(
                    out=nv[:, :, 0], in0=cv[:, :, 0], in1=cv[:, :, 1])
                nc.vector.tensor_tensor(
                    nv[:, :, 1], cv[:, :, 0], cv[:, :, 1],
                    mybir.AluOpType.subtract)
                cur, nxt = nxt, cur
            y = cur.rearrange("p a b -> p (a b)")  # [128, 2048] bf16, = +|y|

            # --- fan-out: out[m, 2048*b + r] = bias[2048*b + r] - y[m, r],
            # all on DVE (gpsimd elementwise contends with DVE for SBUF
            # ports, slowing both ~4x).  Block pairs are one TT with y read
            # twice via a stride-0 AP; pairs alternate between the SWDGE and
            # scalar-HWDGE rings, and the small block-6+tail piece ships
            # last so each chunk's drain ends on a short transfer.
            o = opool.tile([128, OUT_DIM], bf16)
            y2 = bass.AP(y.tensor, y.offset, [y.ap[0], (0, 2), (1, IN_DIM)])
            first, last = c == 0, c == N_CHUNKS - 1
            for k in range(3):
                lo, hi = 2 * k * IN_DIM, 2 * (k + 1) * IN_DIM
                ov = o[:, lo:hi].rearrange("p (two r) -> p two r", two=2)
                bv = bias_ap(lo, hi).rearrange("p (two r) -> p two r", two=2)
                nc.vector.tensor_tensor(ov, bv, y2, mybir.AluOpType.subtract)
                # pair 1 -> scalar ring; pair 2 of the final chunk -> the
                # (by then idle) sync ring so the tail drains 3-wide
                eng = nc.scalar if k == 1 else (
                    nc.sync if (last and k == 2) else nc.gpsimd)
                eng.dma_start(out=out_d[rows, lo:hi], in_=o[:, lo:hi])
            nc.vector.tensor_tensor(
                o[:, DVE_COLS:DVE_COLS + IN_DIM],
                bias_ap(DVE_COLS, DVE_COLS + IN_DIM), y,
                mybir.AluOpType.subtract)
            if last:
                nc.gpsimd.dma_start(
                    out=out_d[rows, DVE_COLS:DVE_COLS + IN_DIM],
                    in_=o[:, DVE_COLS:DVE_COLS + IN_DIM])
            nc.vector.tensor_tensor(
                o[:, DVE_COLS + IN_DIM:],
                bias_ap(DVE_COLS + IN_DIM, OUT_DIM), y[:, :TAIL_COLS],
                mybir.AluOpType.subtract)
            if last:
                nc.scalar.dma_start(out=out_d[rows, DVE_COLS + IN_DIM:],
                                    in_=o[:, DVE_COLS + IN_DIM:])
            else:
                nc.scalar.dma_start(out=out_d[rows, DVE_COLS:],
                                    in_=o[:, DVE_COLS:])

    _split_multiwait_instructions(nc)
    return nc


_PROGRAM = None


def _get_program():
    global _PROGRAM
    if _PROGRAM is None:
        _PROGRAM = _build_program()
    return _PROGRAM


def _run(inputs, trace=False, tmpdir=None):
    from concourse.bass_utils import run_bass_kernel_spmd

    x = np.ascontiguousarray(np.asarray(inputs["x"], dtype=np.float32))
    scale = np.asarray(inputs["scale"], dtype=np.float32)
    bias = np.ascontiguousarray(np.asarray(inputs["bias"], dtype=np.float32))
    assert x.shape == (BATCH, IN_DIM) and bias.shape == (OUT_DIM,)

    h512 = _hadamard(NL)                       # [512, 512]
    hl = np.ascontiguousarray(
        h512.reshape(NP, 128, NL).transpose(1, 0, 2).astype(ml_dtypes.bfloat16))
    biasb = np.ascontiguousarray(np.broadcast_to(
        bias.astype(ml_dtypes.bfloat16)[None, :], (128, OUT_DIM)))
    s = float(scale.reshape(-1)[0])
    a = 1.0 / (NL * s * s)
    sqc = np.ascontiguousarray(
        np.broadcast_to(np.array([a, EPS * a], dtype=np.float32), (128, 2)))

    # xt[kp, c, p, m] = x_shard[c*128 + m, p*128 + kp], bf16
    xb = x.astype(ml_dtypes.bfloat16)
    shards = xb.reshape(N_CORES, N_CHUNKS, 128, 16, 128)
    in_maps = [
        {
            "xt": np.ascontiguousarray(shards[i].transpose(3, 0, 2, 1)),
            "hl": hl,
            "biasb": biasb,
            "sqc": sqc,
        }
        for i in range(N_CORES)
    ]
    nc = _get_program()
    res = run_bass_kernel_spmd(
        nc, in_maps, core_ids=list(range(N_CORES)), trace=trace, tmpdir=tmpdir
    )
    # device emits bf16; upcast to f32 on the host during the gather
    out = np.concatenate(
        [np.asarray(r["out"]).astype(np.float32) for r in res.results], axis=0)
    return out, res


def kernel(x, scale, bias):
    out, _ = _run({"x": x, "scale": scale, "bias": bias})
    return out


# revision 36
# speedup vs baseline: 1.0057x; 1.0057x over previous
"""Trainium2 Bass kernel for nn_HadamardClassifier (self-contained).

Math: out = -scale * l2norm_rows(x) @ H + bias, with H the [2048, 14951]
top-left slice of the 16384x16384 Sylvester Hadamard matrix,
H[i, j] = (-1)^popcount(i & j).

Since row index i < 2048 uses only 11 bits, H[i, j] == H2048[i, j & 2047]:
the output is a periodic tiling of y = xn' @ H2048 (7.3x FLOP reduction).
Further, H2048 = H4 (x) H512 (Kronecker split at bit 9), so
    y[m, jH*512 + jl] = sum_iH H4[iH, jH] * z[m, iH, jl]
    z[m, iH, jl]      = sum_iL H512[iL, jl] * x'[m, iH*512 + iL]

Device dataflow (per core, 512 rows, 4 chunks of 128):
- x arrives PRE-TRANSPOSED and bf16-cast from the host (input marshalling),
  so stage 1 is a straight K=512 PE matmul against H512 panels -- no
  on-device transposes, no PSUM evac of transposes.
- Row norms come free via Parseval: ||z_row||^2 = 512 * ||x_row||^2, so an
  ACT square-accumulate over the PSUM z replaces separate x^2 passes.  The
  positive scale s = scale*sqrt(512)/||z|| is folded into the PSUM
  evacuation; the minus sign is folded into the fan-out by computing
  out = bias - y (tensor_tensor subtract) instead of y' + bias.
- Stage 2 is a 2-stage FWHT over iH on the vector engine in bf16.
- Fan-out: blocks 0-5 subtracted on DVE, block 6 + tail on GpSimd; outputs
  stream over both the SWDGE (gpsimd) and scalar-HWDGE DMA rings in bf16
  (the host upcasts to f32 during the gather -- halves HBM write traffic).
- PSUM z is double-buffered (8 banks) so the PE runs all 64 matmuls as one
  warm back-to-back burst (HAM clock-gate stays open).

Sharding: data-parallel over batch, 8 cores x 512 rows. No collectives.
"""

import numpy as np
import ml_dtypes

BATCH = 4096
IN_DIM = 2048
OUT_DIM = 14951
EPS = 1e-12
N_CORES = 8
M_PER_CORE = BATCH // N_CORES          # 512
N_CHUNKS = M_PER_CORE // 128           # 4 m-chunks of 128 rows
N_FULL_BLOCKS = OUT_DIM // IN_DIM      # 7
TAIL_COLS = OUT_DIM - N_FULL_BLOCKS * IN_DIM  # 615

NH = 4                                 # FWHT size (H4)
NL = IN_DIM // NH                      # 512 = matmul contraction dim
NP = NL // 128                         # 4 128-row panels per iH group

N_B_DVE = 6                            # blocks subtracted on DVE
DVE_COLS = N_B_DVE * IN_DIM            # cols [0, 12288)
GP_COLS = OUT_DIM - DVE_COLS           # cols [12288, 14951): block 6 + tail


def _hadamard(n):
    """Sylvester Hadamard matrix H[i,j] = (-1)^popcount(i&j), float32."""
    i = np.arange(n, dtype=np.uint32)[:, None]
    j = np.arange(n, dtype=np.uint32)[None, :]
    v = i & j
    pc = np.zeros_like(v)
    for b in range(int(n).bit_length()):
        pc += (v >> b) & 1
    return (1.0 - 2.0 * (pc & 1)).astype(np.float32)


def _patch_tile_drain():
    """This walrus build accepts only ONE sync-wait per instruction, but
    Tile's kernel-tail drain attaches the whole global clock to a single
    Drain ('Too many sync wait commands').  Split the waits onto a chain of
    single-wait sequencer nops instead."""
    import concourse.mybir as mybir
    import concourse.tile as tile
    from concourse.vector_clock import ScopedClock

    if getattr(tile.TileContext, "_drain_split_patched", False):
        return

    def _drain_and_barrier(self, tick_clock, wait_clock):
        nc = self.nc
        probe = nc.sync.nop()
        wait_clock.add_sem_waits(
            probe.ins, ScopedClock({None: tick_clock.global_clock})
        )
        si = probe.ins.sync_info
        waits = list(si.on_wait) if si is not None and si.on_wait else []
        if len(waits) > 1:
            si.on_wait = waits[:1]
            for w in waits[1:]:
                n = nc.sync.nop()
                n.ins.sync_info = mybir.SyncInfo(on_wait=[w], on_update=[])
        nc.sync.drain()
        nc.all_engine_barrier()
        assert self.sems is not None
        popped = nc._tile_sem_poison_stack.pop()
        assert popped is self._sem_poison
        nc.clear_and_free_semaphores(list(self.sems.allocated().values()))
        nc.all_engine_barrier()

    tile.TileContext._drain_and_barrier = _drain_and_barrier
    tile.TileContext._drain_split_patched = True


def _split_multiwait_instructions(nc):
    """This walrus build rejects instructions with more than one sync-wait.
    Hoist extra waits onto same-engine nop instructions inserted just before
    the offending instruction (engine queues execute in order, so waiting on
    the nops first is equivalent)."""
    import concourse.mybir as mybir

    n_split = 0
    for blk in nc.m.functions[0].blocks:
        new_list = []
        for inst in blk.instructions:
            si = inst.sync_info
            waits = list(si.on_wait) if si is not None and si.on_wait else []
            if len(waits) > 1:
                for k, w in enumerate(waits[:-1]):
                    nop = mybir.InstNoOp(
                        name=f"{inst.name}-wsplit{k}", ins=[], outs=[])
                    nop.engine = inst.engine
                    nop.sync_info = mybir.SyncInfo(on_wait=[w], on_update=[])
                    new_list.append(nop)
                    n_split += 1
                si.on_wait = waits[-1:]
            new_list.append(inst)
        blk.instructions = new_list
    return n_split


def _build_program():
    import concourse.bass as bass
    import concourse.mybir as mybir
    import concourse.tile as tile

    _patch_tile_drain()
    f32 = mybir.dt.float32
    bf16 = mybir.dt.bfloat16
    nc = bass.Bass()

    # x pre-transposed on the host: xt[kp, c, p, m] = x[c*128+m, p*128+kp]
    xt_d = nc.dram_tensor("xt", [128, N_CHUNKS, 16, 128], bf16,
                          kind="ExternalInput")
    hl_d = nc.dram_tensor("hl", [128, NP, NL], bf16, kind="ExternalInput")
    # bias pre-broadcast to 128 partitions on the host (stride-0 HBM
    # broadcast DMAs crawl at half rate)
    biasb_d = nc.dram_tensor("biasb", [128, OUT_DIM], bf16, kind="ExternalInput")
    # sqrt-input affine: t = sqrt(ss*sqc[0] + sqc[1]) = ||x||/scale, sigma = 1/t
    sqc_d = nc.dram_tensor("sqc", [128, 2], f32, kind="ExternalInput")
    # out in bf16: halves the dominant HBM write traffic; host upcasts to
    # f32 during the gather.
    out_d = nc.dram_tensor("out", [M_PER_CORE, OUT_DIM], bf16,
                           kind="ExternalOutput")

    from contextlib import ExitStack

    with tile.TileContext(nc) as tc, ExitStack() as ctx:
        singles = ctx.enter_context(tc.tile_pool(name="singles", bufs=1))
        scrpool = ctx.enter_context(tc.tile_pool(name="scr", bufs=2))
        fwpool = ctx.enter_context(tc.tile_pool(name="fw", bufs=4))
        opool = ctx.enter_context(tc.tile_pool(name="o", bufs=2))
        z_ps = ctx.enter_context(tc.tile_pool(name="z_ps", bufs=2, space="PSUM"))

        # --- all input loads up-front, issue-parallel across both HWDGE
        # rings, ordered by first use (per-ring DMA completions serialize
        # at ~2.3 us apiece, so each ring carries a short ordered list)
        xts = [None] * N_CHUNKS
        for c in range(N_CHUNKS):
            xts[c] = singles.tile([128, 16, 128], bf16, name=f"xt{c}")
            nc.sync.dma_start(out=xts[c], in_=xt_d[:, c, :, :])
        hl_s = singles.tile([128, NP, NL], bf16)
        nc.scalar.dma_start(out=hl_s, in_=hl_d[:, :, :])
        sqc_s = singles.tile([128, 2], f32)
        nc.scalar.dma_start(out=sqc_s, in_=sqc_d[:, :])
        # bias in 4 separate tiles so consumers only wait for their piece;
        # issues split across the sync and scalar rings
        bias_t = []
        bias_bounds = [0, 4096, 8192, 12288, OUT_DIM]
        for k in range(4):
            lo, hi = bias_bounds[k], bias_bounds[k + 1]
            t = singles.tile([128, hi - lo], bf16, name=f"bias{k}")
            eng = nc.sync if k % 2 == 0 else nc.scalar
            eng.dma_start(out=t, in_=biasb_d[:, lo:hi])
            bias_t.append(t)

        def bias_ap(lo, hi):
            """AP into the bias tiles for global column range [lo, hi)."""
            k = lo // 4096
            base = bias_bounds[k]
            assert hi <= bias_bounds[k + 1]
            return bias_t[k][:, lo - base:hi - base]

        # --- PE prewarm: the engines release from the runtime preamble ~7 us
        # before the first x chunk lands; a few dummy matmuls in that dead
        # window open the HAM clock gate so chunk 0 runs at 2.4 GHz.  They
        # write into the first PSUM buffer, which the first real iH group
        # overwrites (start=True).
        warm_sb = singles.tile([128, 512], bf16)
        nc.vector.memset(warm_sb, 0.0)
        # z in two half-tiles per chunk (iH groups 0-1 / 2-3) so the norm
        # square and the evacuation of the low half overlap the high half's
        # matmuls (Tile dependencies are tile-granular)
        zps = [(z_ps.tile([128, 2 * NL], f32, name="zpl", tag="zpl"),
                z_ps.tile([128, 2 * NL], f32, name="zph", tag="zph"))
               for _ in range(N_CHUNKS)]
        # dummies target chunk 1's tile (WAW never stalls chunk 0)
        for w in range(8):
            nc.tensor.matmul(
                zps[1][0][:, :NL], lhsT=warm_sb[:, :128], rhs=warm_sb,
                start=True, stop=(w == 7))

        for c in range(N_CHUNKS):
            rows = slice(c * 128, (c + 1) * 128)
            zpl, zph = zps[c]

            # --- stage 1: z[m, iH*512+jl] = sum_iL x'[m, iH*512+iL] H512[iL, jl]
            for iH in range(NH):
                zp_half = zpl if iH < 2 else zph
                for p in range(NP):
                    nc.tensor.matmul(
                        zp_half[:, (iH % 2) * NL:(iH % 2 + 1) * NL],
                        lhsT=xts[c][:, iH * NP + p, :], rhs=hl_s[:, p, :],
                        start=(p == 0), stop=(p == NP - 1))

            # --- row norms via Parseval: ss = sum(z^2) = 512*sum(x^2).
            # t = sqrt(ss*sqc0 + sqc1) = ||x||/scale (sqc0 = 1/(512*scale^2)),
            # sigma = 1/t = scale/||x||  (positive; sign folded into subtract).
            # Low-half square overlaps the high half's matmuls.
            sqz = scrpool.tile([128, NH * NL], bf16, tag="sqz")
            ssh = scrpool.tile([128, 2], f32, tag="ssh")
            nc.scalar.activation(
                out=sqz[:, :2 * NL], in_=zpl,
                func=mybir.ActivationFunctionType.Square,
                accum_out=ssh[:, 0:1])
            nc.scalar.activation(
                out=sqz[:, 2 * NL:], in_=zph,
                func=mybir.ActivationFunctionType.Square,
                accum_out=ssh[:, 1:2])
            ss = scrpool.tile([128, 1], f32, tag="ss")
            nc.scalar.add(ss, ssh[:, 0:1], ssh[:, 1:2])
            sig = scrpool.tile([128, 1], f32, tag="sig")
            nc.scalar.activation(
                out=sig, in_=ss, func=mybir.ActivationFunctionType.Sqrt,
                scale=sqc_s[:, 0:1], bias=sqc_s[:, 1:2])
            nc.vector.reciprocal(out=sig, in_=sig)

            # --- evacuate PSUM with the positive row scale, cast bf16.
            # Halves evac separately so FWHT stage 1's first butterfly
            # (which pairs iH 0 and 1 = the low half) starts after evac_lo.
            zw0 = fwpool.tile([128, NH, NL], bf16, tag="zw0")
            zw1 = fwpool.tile([128, NH, NL], bf16, tag="zw1")
            zw0f = zw0.rearrange("p a b -> p (a b)")
            nc.scalar.activation(
                out=zw0f[:, :2 * NL], in_=zpl,
                func=mybir.ActivationFunctionType.Copy, scale=sig)
            nc.scalar.activation(
                out=zw0f[:, 2 * NL:], in_=zph,
                func=mybir.ActivationFunctionType.Copy, scale=sig)

            # --- stage 2: FWHT over iH (dim 1), 2 butterfly stages on DVE ---
            cur, nxt = zw0, zw1
            for s in range(NH.bit_length() - 1):
                t = 1 << s
                cv = cur.rearrange("p (g two t) jl -> p g two t jl", two=2, t=t)
                nv = nxt.rearrange("p (g two t) jl -> p g two t jl", two=2, t=t)
                nc.vector.tensor_add(
                    out=nv[:, :, 0], in0=cv[:, :, 0], in1=cv[:, :, 1])
                nc.vector.tensor_tensor(
                    nv[:, :, 1], cv[:, :, 0], cv[:, :, 1],
                    mybir.AluOpType.subtract)
                cur, nxt = nxt, cur
            y = cur.rearrange("p a b -> p (a b)")  # [128, 2048] bf16, = +|y|

            # --- fan-out: out[m, 2048*b + r] = bias[2048*b + r] - y[m, r],
            # all on DVE (gpsimd elementwise contends with DVE for SBUF
            # ports, slowing both ~4x).  Block pairs are one TT with y read
            # twice via a stride-0 AP; pairs alternate between the SWDGE and
            # scalar-HWDGE rings, and the small block-6+tail piece ships
            # last so each chunk's drain ends on a short transfer.
            o = opool.tile([128, OUT_DIM], bf16)
            y2 = bass.AP(y.tensor, y.offset, [y.ap[0], (0, 2), (1, IN_DIM)])
            first, last = c == 0, c == N_CHUNKS - 1
            for k in range(3):
                lo, hi = 2 * k * IN_DIM, 2 * (k + 1) * IN_DIM
                ov = o[:, lo:hi].rearrange("p (two r) -> p two r", two=2)
                bv = bias_ap(lo, hi).rearrange("p (two r) -> p two r", two=2)
                nc.vector.tensor_tensor(ov, bv, y2, mybir.AluOpType.subtract)
                eng = nc.scalar if k == 1 else nc.gpsimd
                eng.dma_start(out=out_d[rows, lo:hi], in_=o[:, lo:hi])
            nc.vector.tensor_tensor(
                o[:, DVE_COLS:DVE_COLS + IN_DIM],
                bias_ap(DVE_COLS, DVE_COLS + IN_DIM), y,
                mybir.AluOpType.subtract)
            if last:
                nc.gpsimd.dma_start(
                    out=out_d[rows, DVE_COLS:DVE_COLS + IN_DIM],
                    in_=o[:, DVE_COLS:DVE_COLS + IN_DIM])
            nc.vector.tensor_tensor(
                o[:, DVE_COLS + IN_DIM:],
                bias_ap(DVE_COLS + IN_DIM, OUT_DIM), y[:, :TAIL_COLS],
                mybir.AluOpType.subtract)
            if last:
                nc.scalar.dma_start(out=out_d[rows, DVE_COLS + IN_DIM:],
                                    in_=o[:, DVE_COLS + IN_DIM:])
            else:
                nc.scalar.dma_start(out=out_d[rows, DVE_COLS:],
                                    in_=o[:, DVE_COLS:])

    _split_multiwait_instructions(nc)
    return nc


_PROGRAM = None


def _get_program():
    global _PROGRAM
    if _PROGRAM is None:
        _PROGRAM = _build_program()
    return _PROGRAM


def _run(inputs, trace=False, tmpdir=None):
    from concourse.bass_utils import run_bass_kernel_spmd

    x = np.ascontiguousarray(np.asarray(inputs["x"], dtype=np.float32))
    scale = np.asarray(inputs["scale"], dtype=np.float32)
    bias = np.ascontiguousarray(np.asarray(inputs["bias"], dtype=np.float32))
    assert x.shape == (BATCH, IN_DIM) and bias.shape == (OUT_DIM,)

    h512 = _hadamard(NL)                       # [512, 512]
    hl = np.ascontiguousarray(
        h512.reshape(NP, 128, NL).transpose(1, 0, 2).astype(ml_dtypes.bfloat16))
    biasb = np.ascontiguousarray(np.broadcast_to(
        bias.astype(ml_dtypes.bfloat16)[None, :], (128, OUT_DIM)))
    s = float(scale.reshape(-1)[0])
    a = 1.0 / (NL * s * s)
    sqc = np.ascontiguousarray(
        np.broadcast_to(np.array([a, EPS * a], dtype=np.float32), (128, 2)))

    # xt[kp, c, p, m] = x_shard[c*128 + m, p*128 + kp], bf16
    xb = x.astype(ml_dtypes.bfloat16)
    shards = xb.reshape(N_CORES, N_CHUNKS, 128, 16, 128)
    in_maps = [
        {
            "xt": np.ascontiguousarray(shards[i].transpose(3, 0, 2, 1)),
            "hl": hl,
            "biasb": biasb,
            "sqc": sqc,
        }
        for i in range(N_CORES)
    ]
    nc = _get_program()
    res = run_bass_kernel_spmd(
        nc, in_maps, core_ids=list(range(N_CORES)), trace=trace, tmpdir=tmpdir
    )
    # device emits bf16; upcast to f32 on the host during the gather
    out = np.concatenate(
        [np.asarray(r["out"]).astype(np.float32) for r in res.results], axis=0)
    return out, res


def kernel(x, scale, bias):
    out, _ = _run({"x": x, "scale": scale, "bias": bias})
    return out
